# revision 1
# baseline (speedup 1.0000x reference)
"""Trainium2 Bass kernel for nn_ConditionalMLN.

Math: the reference reduces exactly (cart.sum(-1) == 1 algebraically) to
    out = sum_r w_r * (G + cnt_r - S_r),   S_r = sum_g flag[r,g] * Z[r,g]
    Z = prod_k t_k,  t_k = select(mask_k, p[i_k], 1 - p[i_k])
        = sigma_k * (p[i_k] + m_k - 1),  sigma_k = 2*m_k - 1
so each NeuronCore computes S_r for its 2 rules (R=16 sharded over 8 cores)
via 1.2M table gathers + elementwise products + a reduction.

Gather: per-element indirect DMA (SWDGE row-mode: 128 offsets -> 128 scalar
descriptors per instruction), which is the only per-element gather primitive
that compiles and runs correctly on this toolchain.
"""

import numpy as np

R, G, K, N = 16, 200000, 3, 2000000
NCORES = 8
P = 128
RLOC = R // NCORES            # rules per core
GCOLS = (G + P - 1) // P      # 1563 columns per rule (G padded to 200064)
GPAD = GCOLS * P
COLS = RLOC * GCOLS           # 3126 columns per core

_CACHE = {}


def _build_program():
    from concourse import bass, mybir

    nc = bass.Bass("TRN2", target_bir_lowering=False, debug=False,
                   num_devices=NCORES)

    table = nc.declare_dram_parameter("table", [N, 1], mybir.dt.float32,
                                      isOutput=False)
    idx_d = [nc.declare_dram_parameter(f"idx{k}", [P, COLS], mybir.dt.int32,
                                       isOutput=False) for k in range(K)]
    msk_d = [nc.declare_dram_parameter(f"msk{k}", [P, COLS], mybir.dt.int8,
                                       isOutput=False) for k in range(K)]
    flg_d = nc.declare_dram_parameter("flg", [P, COLS], mybir.dt.int8,
                                      isOutput=False)
    y_d = nc.declare_dram_parameter("y", [P, RLOC], mybir.dt.float32,
                                    isOutput=True)

    f32, i32, i8 = mybir.dt.float32, mybir.dt.int32, mybir.dt.int8
    idx_s = [nc.alloc_sbuf_tensor(f"idx{k}_s", [P, COLS], i32) for k in range(K)]
    msk_s = [nc.alloc_sbuf_tensor(f"msk{k}_s", [P, COLS], i8) for k in range(K)]
    flg_s = nc.alloc_sbuf_tensor("flg_s", [P, COLS], i8)
    p_s = [nc.alloc_sbuf_tensor(f"p{k}_s", [P, COLS], f32) for k in range(K)]
    mf_s = nc.alloc_sbuf_tensor("mf_s", [P, COLS], f32)
    sg_s = nc.alloc_sbuf_tensor("sg_s", [P, COLS], f32)
    z_s = nc.alloc_sbuf_tensor("z_s", [P, COLS], f32)
    acc_s = nc.alloc_sbuf_tensor("acc_s", [P, RLOC], f32)

    NDMA_IN = 2 * K + 1
    AluOp = mybir.AluOpType

    with (
        nc.Block() as block,
        nc.semaphore("dsem") as dsem,
        nc.semaphore("gsem") as gsem,
        nc.semaphore("vsem") as vsem,
        nc.semaphore("osem") as osem,
    ):

        @block.sync
        def _(sync):
            for k in range(K):
                sync.dma_start(out=idx_s[k].ap(), in_=idx_d[k][:]).then_inc(dsem, 16)
            for k in range(K):
                sync.dma_start(out=msk_s[k].ap(), in_=msk_d[k][:]).then_inc(dsem, 16)
            sync.dma_start(out=flg_s.ap(), in_=flg_d[:]).then_inc(dsem, 16)
            sync.wait_ge(vsem, 1)
            sync.dma_start(out=y_d[:], in_=acc_s.ap()).then_inc(osem, 16)
            sync.wait_ge(osem, 16)

        @block.gpsimd
        def _(g):
            g.wait_ge(dsem, 16 * K)  # idx planes resident
            for k in range(K):
                ip = idx_s[k].ap()
                op = p_s[k].ap()
                for j in range(COLS):
                    g.indirect_dma_start(
                        out=op[:, j:j + 1],
                        out_offset=None,
                        in_=table[:],
                        in_offset=bass.IndirectOffsetOnAxis(
                            ap=ip[:, j:j + 1], axis=0),
                    ).then_inc(gsem, 16)

        @block.vector
        def _(v):
            v.wait_ge(dsem, 16 * NDMA_IN)
            v.wait_ge(gsem, 16 * K * COLS)
            mf = mf_s.ap()
            sg = sg_s.ap()
            z = z_s.ap()
            for k in range(K):
                p = p_s[k].ap()
                v.tensor_copy(mf, msk_s[k].ap())            # int8 -> f32
                # d_k = p + m - 1   (stored in p plane)
                v.tensor_tensor(out=p, in0=p, in1=mf, op=AluOp.add)
                v.tensor_scalar(out=p, in0=p, scalar1=-1.0, scalar2=None,
                                op0=AluOp.add)
                # sigma_k = 2m - 1 ; accumulate product of sigmas in sg
                v.tensor_scalar(out=mf, in0=mf, scalar1=2.0, scalar2=-1.0,
                                op0=AluOp.mult, op1=AluOp.add)
                if k == 0:
                    v.tensor_copy(sg, mf)
                else:
                    v.tensor_tensor(out=sg, in0=sg, in1=mf, op=AluOp.mult)
            # z = d0*d1*d2 * sg * flag
            v.tensor_tensor(out=z, in0=p_s[0].ap(), in1=p_s[1].ap(), op=AluOp.mult)
            v.tensor_tensor(out=z, in0=z, in1=p_s[2].ap(), op=AluOp.mult)
            v.tensor_tensor(out=z, in0=z, in1=sg, op=AluOp.mult)
            v.tensor_copy(mf, flg_s.ap())                   # int8 -> f32
            v.tensor_tensor(out=z, in0=z, in1=mf, op=AluOp.mult)
            for r in range(RLOC):
                red = v.tensor_reduce(
                    acc_s.ap()[:, r:r + 1],
                    z[:, r * GCOLS:(r + 1) * GCOLS],
                    mybir.AxisListType.X,
                    AluOp.add,
                )
            red.then_inc(vsem, 1)

    return nc


def _layout(a):
    """[RLOC, G] -> [P, COLS] with element (r, g) at [g % P, r*GCOLS + g//P]."""
    rloc = a.shape[0]
    pad = np.zeros((rloc, GPAD - G), dtype=a.dtype)
    ap = np.concatenate([a, pad], axis=1)          # [RLOC, GPAD]
    ap = ap.reshape(rloc, GCOLS, P)                # [r, col, part]
    ap = np.transpose(ap, (2, 0, 1)).reshape(P, rloc * GCOLS)
    return np.ascontiguousarray(ap)


def kernel(posterior_prob, observed_rule_cnts, rule_weights,
           latent_var_inds, latent_neg_mask, obs_zero_flag):
    posterior_prob = np.asarray(posterior_prob)
    observed_rule_cnts = np.asarray(observed_rule_cnts)
    rule_weights = np.asarray(rule_weights)
    latent_var_inds = np.asarray(latent_var_inds)
    latent_neg_mask = np.asarray(latent_neg_mask)
    obs_zero_flag = np.asarray(obs_zero_flag)

    if "nc" not in _CACHE:
        _CACHE["nc"] = _build_program()
    nc = _CACHE["nc"]

    table = np.ascontiguousarray(posterior_prob.astype(np.float32).reshape(N, 1))
    in_maps = []
    for c in range(NCORES):
        rules = slice(RLOC * c, RLOC * (c + 1))
        m = {"table": table, "flg": _layout(
            obs_zero_flag[rules].astype(np.int8))}
        for k in range(K):
            m[f"idx{k}"] = _layout(
                latent_var_inds[rules, :, k].astype(np.int32))
            m[f"msk{k}"] = _layout(
                latent_neg_mask[rules, :, k].astype(np.int8))
        in_maps.append(m)

    from concourse.bass_utils import run_bass_kernel_spmd
    res = run_bass_kernel_spmd(nc, in_maps, core_ids=list(range(NCORES)))

    s = np.empty(R, dtype=np.float64)
    for c in range(NCORES):
        part = res.results[c]["y"].sum(axis=0)      # [RLOC]
        s[RLOC * c:RLOC * (c + 1)] = part
    scores = np.float64(G) + observed_rule_cnts.astype(np.float64) - s
    out = rule_weights.astype(np.float64) @ scores
    return np.asarray([out], dtype=np.float32)



# revision 2
# speedup vs baseline: 2.3971x; 2.3971x over previous
"""Trainium2 Bass kernel for nn_ConditionalMLN.

Math: the reference reduces exactly (cart.sum(-1) == 1 algebraically) to
    out = sum_r w_r * (G + cnt_r - S_r),   S_r = sum_g flag[r,g] * Z[r,g]
    Z = prod_k t_k,  t_k = select(mask_k, p[i_k], 1 - p[i_k])

Host-side folding: build a double table  table2[2i] = 1 - p[i],
table2[2i+1] = p[i], table2[2N] = 0.0  and transformed indices
    idx''[r,g,k] = 2*i + m            if flag[r,g]
                 = 2N  (-> 0.0)       otherwise (and for g-padding)
so the device only needs:  gather t_k = table2[idx''_k]  (3 planes),
z = t0*t1*t2, per-rule sum.  No mask/flag traffic, no select math.

Gather: batched indirect DMA (SWDGE) - one instruction per 64-column
chunk of a [128, COLS] plane carries ~8192 scalar descriptors, which
amortizes the ~1us fixed SWDGE overhead that dominated the baseline
(which issued one instruction per 128 offsets).
"""

import numpy as np

R, G, K, N = 16, 200000, 3, 2000000
NCORES = 8
P = 128
RLOC = R // NCORES            # rules per core
GCOLS = (G + P - 1) // P      # 1563 columns per rule (G padded to 200064)
GPAD = GCOLS * P
COLS = RLOC * GCOLS           # 3126 columns per core
NT = 2 * N + 1                # double table + zero entry
ZIDX = 2 * N                  # index of the 0.0 entry
CHUNK = 64                    # columns per indirect DMA (~8192 descriptors)

_CACHE = {}


def _chunks():
    out = []
    j = 0
    while j < COLS:
        out.append((j, min(j + CHUNK, COLS)))
        j += CHUNK
    return out


def _build_program():
    from concourse import bass, mybir

    nc = bass.Bass("TRN2", target_bir_lowering=False, debug=False,
                   num_devices=NCORES)

    table = nc.declare_dram_parameter("table2", [NT, 1], mybir.dt.float32,
                                      isOutput=False)
    idx_d = [nc.declare_dram_parameter(f"idx{k}", [P, COLS], mybir.dt.int32,
                                       isOutput=False) for k in range(K)]
    y_d = nc.declare_dram_parameter("y", [P, RLOC], mybir.dt.float32,
                                    isOutput=True)

    f32, i32 = mybir.dt.float32, mybir.dt.int32
    idx_s = [nc.alloc_sbuf_tensor(f"idx{k}_s", [P, COLS], i32) for k in range(K)]
    p_s = [nc.alloc_sbuf_tensor(f"p{k}_s", [P, COLS], f32) for k in range(K)]
    z_s = nc.alloc_sbuf_tensor("z_s", [P, COLS], f32)
    acc_s = nc.alloc_sbuf_tensor("acc_s", [P, RLOC], f32)

    AluOp = mybir.AluOpType
    chunks = _chunks()

    with (
        nc.Block() as block,
        nc.semaphore("dsem") as dsem,
        nc.semaphore("gsem") as gsem,
        nc.semaphore("vsem") as vsem,
        nc.semaphore("osem") as osem,
    ):

        @block.sync
        def _(sync):
            for k in range(K):
                sync.dma_start(out=idx_s[k].ap(), in_=idx_d[k][:]).then_inc(dsem, 16)
            sync.wait_ge(vsem, 1)
            sync.dma_start(out=y_d[:], in_=acc_s.ap()).then_inc(osem, 16)
            sync.wait_ge(osem, 16)

        @block.gpsimd
        def _(g):
            g.wait_ge(dsem, 16 * K)  # idx planes resident
            for j0, j1 in chunks:
                for k in range(K):
                    g.indirect_dma_start(
                        out=p_s[k].ap()[:, j0:j1],
                        out_offset=None,
                        in_=table[:],
                        in_offset=bass.IndirectOffsetOnAxis(
                            ap=idx_s[k].ap()[:, j0:j1], axis=0),
                    ).then_inc(gsem, 16)

        @block.vector
        def _(v):
            z = z_s.ap()
            for c, (j0, j1) in enumerate(chunks):
                v.wait_ge(gsem, 16 * K * (c + 1))
                v.tensor_tensor(out=z[:, j0:j1], in0=p_s[0].ap()[:, j0:j1],
                                in1=p_s[1].ap()[:, j0:j1], op=AluOp.mult)
                v.tensor_tensor(out=z[:, j0:j1], in0=z[:, j0:j1],
                                in1=p_s[2].ap()[:, j0:j1], op=AluOp.mult)
            for r in range(RLOC):
                red = v.tensor_reduce(
                    acc_s.ap()[:, r:r + 1],
                    z[:, r * GCOLS:(r + 1) * GCOLS],
                    mybir.AxisListType.X,
                    AluOp.add,
                )
            red.then_inc(vsem, 1)

    return nc


def _layout(a, pad_value):
    """[RLOC, G] -> [P, COLS] with element (r, g) at [g % P, r*GCOLS + g//P]."""
    rloc = a.shape[0]
    pad = np.full((rloc, GPAD - G), pad_value, dtype=a.dtype)
    ap = np.concatenate([a, pad], axis=1)          # [RLOC, GPAD]
    ap = ap.reshape(rloc, GCOLS, P)                # [r, col, part]
    ap = np.transpose(ap, (2, 0, 1)).reshape(P, rloc * GCOLS)
    return np.ascontiguousarray(ap)


def build_in_maps(posterior_prob, latent_var_inds, latent_neg_mask,
                  obs_zero_flag):
    p = np.asarray(posterior_prob).astype(np.float32).ravel()
    t2 = np.empty((NT, 1), dtype=np.float32)
    t2[0:2 * N:2, 0] = 1.0 - p
    t2[1:2 * N:2, 0] = p
    t2[ZIDX, 0] = 0.0

    inds = np.asarray(latent_var_inds).astype(np.int64)
    mask = np.asarray(latent_neg_mask).astype(np.int64)
    flag = np.asarray(obs_zero_flag).astype(bool)
    idx2 = 2 * inds + mask                          # [R, G, K]
    idx2 = np.where(flag[:, :, None], idx2, ZIDX).astype(np.int32)

    in_maps = []
    for c in range(NCORES):
        rules = slice(RLOC * c, RLOC * (c + 1))
        m = {"table2": t2}
        for k in range(K):
            m[f"idx{k}"] = _layout(idx2[rules, :, k], ZIDX)
        in_maps.append(m)
    return in_maps


def kernel(posterior_prob, observed_rule_cnts, rule_weights,
           latent_var_inds, latent_neg_mask, obs_zero_flag):
    observed_rule_cnts = np.asarray(observed_rule_cnts)
    rule_weights = np.asarray(rule_weights)

    if "nc" not in _CACHE:
        _CACHE["nc"] = _build_program()
    nc = _CACHE["nc"]

    in_maps = build_in_maps(posterior_prob, latent_var_inds,
                            latent_neg_mask, obs_zero_flag)

    from concourse.bass_utils import run_bass_kernel_spmd
    res = run_bass_kernel_spmd(nc, in_maps, core_ids=list(range(NCORES)))

    s = np.empty(R, dtype=np.float64)
    for c in range(NCORES):
        part = res.results[c]["y"].astype(np.float64).sum(axis=0)   # [RLOC]
        s[RLOC * c:RLOC * (c + 1)] = part
    scores = np.float64(G) + observed_rule_cnts.astype(np.float64) - s
    out = rule_weights.astype(np.float64) @ scores
    return np.asarray([out], dtype=np.float32)
